# revision 23
# baseline (speedup 1.0000x reference)
"""Self-contained MHA kernel for Trainium2, 8 NeuronCores.

Problem: B=4, T=2048, D=1024, H=16 causal MHA, fp32, no bias.
Sharding: core c handles batch b=c//2 and head-group hg=c%2 (8 heads each),
Megatron-style: Wq/Wk/Wv column-sharded, Wo row-sharded; host sums the two
partial outputs per batch.

Per-core pipeline (emission interleaved so ACT-bound attention overlaps
PE-bound projections of the next head-pair):
 - PE-transpose x -> xT [d, t] (f32r), split in two t-group tiles
 - weight-stationary projections: QT/KT [2 heads on partitions, t],
   V via V^T + PE transpose, stored as [t, A|ones|B|ones] (double-buffered)
 - attention per head-pair, transposed scores S^T[k, q] = K Q^T
   (row-packed 2 heads per PE pass), exp on ScalarE with scale=1/8,
   no max subtraction (scores ~ N(0,1)); fully-masked columns skipped,
   triangular mask multiply only on the diagonal 128-col block
 - ctx^T[dv,q] + softmax denominator in one matmul: lhsT=[V|ones]
 - prompt PSUM evacuation, then reciprocal + partition_broadcast + multiply
 - out projection: lhsT=ctx tiles, rhs=WoT -> out[t, e] partial, interleaved
   with the last pair's attention
"""

import numpy as np

B, T, D, H = 4, 2048, 1024, 16
DK = 64
NCORES = 8
NPAIR = 4        # head-pairs per core
ESH = 512        # output-feature shard per core (8 heads * 64)
RO = 4           # ones columns appended per head in the V stationary
VW = 2 * (DK + RO)   # 136 cols per t-tile in V

_nc_cache = None


def _build():
    global _nc_cache
    if _nc_cache is not None:
        return _nc_cache

    from contextlib import ExitStack

    import concourse.bacc as bacc
    import concourse.mybir as mybir
    import concourse.tile as tile
    from concourse.masks import make_identity, make_upper_triangular

    F32 = mybir.dt.float32
    F32R = mybir.dt.float32r
    AF = mybir.ActivationFunctionType
    OP = mybir.AluOpType

    nc = bacc.Bacc("TRN2", target_bir_lowering=False, debug=False,
                   num_devices=NCORES)
    x_d = nc.declare_dram_parameter("x", [T, D], F32, isOutput=False)
    wqt_d = nc.declare_dram_parameter("wqt", [D, ESH], F32, isOutput=False)
    wkt_d = nc.declare_dram_parameter("wkt", [D, ESH], F32, isOutput=False)
    wvt_d = nc.declare_dram_parameter("wvt", [D, ESH], F32, isOutput=False)
    wot_d = nc.declare_dram_parameter("wot", [ESH, D], F32, isOutput=False)
    out_d = nc.declare_dram_parameter("out", [T, D], F32, isOutput=True)

    with tile.TileContext(nc) as tc, ExitStack() as ctx:
        const_p = ctx.enter_context(tc.tile_pool(name="const", bufs=1))
        xt_p = ctx.enter_context(tc.tile_pool(name="xt", bufs=1))
        xin_p = ctx.enter_context(tc.tile_pool(name="xin", bufs=3))
        wt_p = ctx.enter_context(tc.tile_pool(name="wt", bufs=4))
        qt_p = ctx.enter_context(tc.tile_pool(name="qt", bufs=2))
        kt_p = ctx.enter_context(tc.tile_pool(name="kt", bufs=2))
        v_pool = ctx.enter_context(tc.tile_pool(name="vp", bufs=2))
        vt_p = ctx.enter_context(tc.tile_pool(name="vtmp", bufs=1))
        ctx_p = ctx.enter_context(tc.tile_pool(name="ctxp", bufs=1))
        s_p = ctx.enter_context(tc.tile_pool(name="sp", bufs=3))
        st_p = ctx.enter_context(tc.tile_pool(name="stage", bufs=2))
        bc_p = ctx.enter_context(tc.tile_pool(name="bc", bufs=1))
        rec_p = ctx.enter_context(tc.tile_pool(name="rec", bufs=1))
        psum_s = ctx.enter_context(tc.tile_pool(name="psum_s", bufs=2,
                                                space="PSUM"))
        psum_ctx = ctx.enter_context(tc.tile_pool(name="psum_ctx", bufs=2,
                                                  space="PSUM"))
        psum_mm = ctx.enter_context(tc.tile_pool(name="psum_mm", bufs=1,
                                                 space="PSUM"))

        # constants
        ident = const_p.tile([128, 128], F32)
        make_identity(nc, ident[:])
        trimask = const_p.tile([128, 128], F32)
        make_upper_triangular(nc, trimask[:], val=1.0, diag=True)
        ones32 = const_p.tile([128, RO], F32)
        nc.gpsimd.memset(ones32[:], 1.0)

        # xT split per t-group for finer dependencies
        xT = [xt_p.tile([128, 8, 1024], F32R, tag=f"xt{g}", name=f"xT{g}")
              for g in range(2)]
        ctx_sb = [ctx_p.tile([128, T], F32R, tag=f"ctx{p}", name=f"ctx_sb{p}")
                  for p in range(NPAIR)]

        W_TILES = {}   # p -> (wq, wk, wv)
        QKV = {}       # p -> (QT, KT, V)
        WO = {}
        VT_TMP = {}

        def emit_a0_group(tg):
            """x -> xT for t-tiles 8*tg .. 8*tg+7."""
            for tt in range(8 * tg, 8 * (tg + 1)):
                xin = xin_p.tile([128, D], F32, tag="xin", name=f"xin{tt}")
                nc.sync.dma_start(out=xin[:],
                                  in_=x_d[tt * 128:(tt + 1) * 128, :])
                for dg in range(2):
                    pst = psum_s.tile([128, 4, 128], F32, tag="s",
                                      name=f"pst{tt}_{dg}")
                    for j in range(4):
                        dt_ = dg * 4 + j
                        nc.tensor.matmul(pst[:, j, :],
                                         xin[:, dt_ * 128:(dt_ + 1) * 128],
                                         ident[:], is_transpose=True,
                                         start=(j == 0), stop=(j == 3))
                    lo = (tt - 8 * tg) * 128
                    nc.vector.tensor_copy(
                        xT[tg][:, dg * 4:(dg + 1) * 4, lo:lo + 128], pst[:])

        def emit_w_dma(p):
            wq = wt_p.tile([128, 8, 128], F32R, tag="w", name=f"wq{p}")
            wk = wt_p.tile([128, 8, 128], F32R, tag="w", name=f"wk{p}")
            wv = wt_p.tile([128, 8, 128], F32R, tag="w", name=f"wv{p}")
            esl = slice(p * 128, (p + 1) * 128)
            for dt_ in range(8):
                dsl = slice(dt_ * 128, (dt_ + 1) * 128)
                nc.sync.dma_start(out=wq[:, dt_, :],
                                  in_=wqt_d[dsl, esl].bitcast(F32R))
                nc.sync.dma_start(out=wk[:, dt_, :],
                                  in_=wkt_d[dsl, esl].bitcast(F32R))
                nc.sync.dma_start(out=wv[:, dt_, :],
                                  in_=wvt_d[dsl, esl].bitcast(F32R))
            W_TILES[p] = (wq, wk, wv)

        def alloc_qkv(p):
            QT = qt_p.tile([128, T], F32R, tag="q", name=f"QT{p}")
            KT = kt_p.tile([128, T], F32R, tag="k", name=f"KT{p}")
            V = v_pool.tile([128, 16, VW], F32R, tag="v", name=f"V{p}")
            QKV[p] = (QT, KT, V)

        def _qkv_mm_unit(p, tg, which):
            """one 16-MM projection group + evac (which in 'q','k','v')."""
            wq, wk, wv = W_TILES[p]
            QT, KT, V = QKV[p]
            w_sb, dst = {"q": (wq, QT), "k": (wk, KT), "v": (wv, None)}[which]
            ps = psum_mm.tile([128, 2, 512], F32, tag="mm",
                              name=f"ps{p}_{tg}_{which}")
            for half in range(2):
                csl = slice(half * 512, (half + 1) * 512)
                for dt_ in range(8):
                    nc.tensor.matmul(ps[:, half, :], w_sb[:, dt_, :],
                                     xT[tg][:, dt_, csl],
                                     start=(dt_ == 0), stop=(dt_ == 7))
            if which == "v":
                vtmp = vt_p.tile([128, 1024], F32, tag="vt",
                                 name=f"vtmp{p}_{tg}")
                nc.vector.tensor_copy(vtmp[:],
                                      ps[:].rearrange("p a b -> p (a b)"))
                VT_TMP[(p, tg)] = vtmp
            else:
                nc.vector.tensor_copy(
                    dst[:, tg * 1024:(tg + 1) * 1024],
                    ps[:].rearrange("p a b -> p (a b)"))

        def _v_tr_unit(p, tg, vg):
            """4 V transposes + layout copies."""
            V = QKV[p][2]
            vtmp = VT_TMP[(p, tg)]
            pst = psum_s.tile([128, 4, 128], F32, tag="s",
                              name=f"pstv{p}_{tg}_{vg}")
            for j in range(4):
                sub = vg * 4 + j
                nc.tensor.matmul(
                    pst[:, j, :],
                    vtmp[:, sub * 128:(sub + 1) * 128],
                    ident[:], is_transpose=True,
                    start=(j == 0), stop=(j == 3))
            gts = slice(tg * 8 + vg * 4, tg * 8 + (vg + 1) * 4)
            nc.vector.tensor_copy(V[:, gts, 0:DK], pst[:, :, 0:DK])
            nc.vector.tensor_copy(V[:, gts, DK + RO:DK + RO + DK],
                                  pst[:, :, DK:128])
            nc.vector.tensor_copy(
                V[:, gts, DK:DK + RO],
                ones32[:].unsqueeze(1).broadcast_to([128, 4, RO]))
            nc.vector.tensor_copy(
                V[:, gts, DK + RO + DK:VW],
                ones32[:].unsqueeze(1).broadcast_to([128, 4, RO]))

        def qkv_units(p):
            units = []
            for tg in range(2):
                for which in ("q", "k", "v"):
                    units.append(lambda p=p, tg=tg, w=which: _qkv_mm_unit(p, tg, w))
                for vg in range(2):
                    units.append(lambda p=p, tg=tg, vg=vg: _v_tr_unit(p, tg, vg))
            return units

        def emit_qkv_piece(p, piece):
            """piece 0..3: (tg0:QK, tg0:V, tg1:QK, tg1:V)."""
            tg, kind = divmod(piece, 2)
            if kind == 0:
                _qkv_mm_unit(p, tg, "q")
                _qkv_mm_unit(p, tg, "k")
            else:
                _qkv_mm_unit(p, tg, "v")
                _v_tr_unit(p, tg, 0)
                _v_tr_unit(p, tg, 1)

        def emit_qkv_chunk(p, ch):
            """512-col chunk granularity (used for pair 0 startup)."""
            wq, wk, wv = W_TILES[p]
            QT, KT, V = QKV[p]
            tg, half = divmod(ch, 2)
            csl = slice(half * 512, (half + 1) * 512)
            osl = slice(ch * 512, (ch + 1) * 512)
            for w_sb, dst in ((wq, QT), (wk, KT)):
                ps = psum_mm.tile([128, 512], F32, tag="mm",
                                  name=f"psc{p}_{ch}_{dst.name}")
                for dt_ in range(8):
                    nc.tensor.matmul(ps[:], w_sb[:, dt_, :],
                                     xT[tg][:, dt_, csl],
                                     start=(dt_ == 0), stop=(dt_ == 7))
                nc.vector.tensor_copy(dst[:, osl], ps[:])
            psv = psum_mm.tile([128, 512], F32, tag="mm", name=f"psvc{p}_{ch}")
            for dt_ in range(8):
                nc.tensor.matmul(psv[:], wv[:, dt_, :], xT[tg][:, dt_, csl],
                                 start=(dt_ == 0), stop=(dt_ == 7))
            vtmp = vt_p.tile([128, 512], F32, tag="vt", name=f"vtc{p}_{ch}")
            nc.vector.tensor_copy(vtmp[:], psv[:])
            pst = psum_s.tile([128, 4, 128], F32, tag="s", name=f"pstc{p}_{ch}")
            for j in range(4):
                nc.tensor.matmul(pst[:, j, :],
                                 vtmp[:, j * 128:(j + 1) * 128],
                                 ident[:], is_transpose=True,
                                 start=(j == 0), stop=(j == 3))
            gts = slice(ch * 4, (ch + 1) * 4)
            nc.vector.tensor_copy(V[:, gts, 0:DK], pst[:, :, 0:DK])
            nc.vector.tensor_copy(V[:, gts, DK + RO:DK + RO + DK],
                                  pst[:, :, DK:128])
            nc.vector.tensor_copy(
                V[:, gts, DK:DK + RO],
                ones32[:].unsqueeze(1).broadcast_to([128, 4, RO]))
            nc.vector.tensor_copy(
                V[:, gts, DK + RO + DK:VW],
                ones32[:].unsqueeze(1).broadcast_to([128, 4, RO]))

        def emit_a0_chunk(ch):
            """x -> xT for t-tiles 4*ch .. 4*ch+3."""
            tg = ch // 2
            for tt in range(4 * ch, 4 * (ch + 1)):
                xin = xin_p.tile([128, D], F32, tag="xin", name=f"xin{tt}")
                nc.sync.dma_start(out=xin[:],
                                  in_=x_d[tt * 128:(tt + 1) * 128, :])
                for dg in range(2):
                    pst = psum_s.tile([128, 4, 128], F32, tag="s",
                                      name=f"pst{tt}_{dg}")
                    for j in range(4):
                        dt_ = dg * 4 + j
                        nc.tensor.matmul(pst[:, j, :],
                                         xin[:, dt_ * 128:(dt_ + 1) * 128],
                                         ident[:], is_transpose=True,
                                         start=(j == 0), stop=(j == 3))
                    lo = (tt - 8 * tg) * 128
                    nc.vector.tensor_copy(
                        xT[tg][:, dg * 4:(dg + 1) * 4, lo:lo + 128], pst[:])

        def emit_attention(p, qc, filler=None):
            QT, KT, V = QKV[p]
            qsl = slice(qc * 512, (qc + 1) * 512)
            nki = 4 * (qc + 1)
            psA = psum_ctx.tile([DK + RO, 512], F32, tag="ctx",
                                name=f"psA{p}_{qc}")
            psB = psum_ctx.tile([DK + RO, 512], F32, tag="ctx",
                                name=f"psB{p}_{qc}")
            for ki in range(nki):
                if filler and ki > 0 and (ki * len(filler._units)) % nki == 0:
                    filler.pop()
                ksl = slice(ki * 128, (ki + 1) * 128)
                ci = max(0, (ki - 4 * qc) * 128)
                cq = min(ci, 256)     # keep QK matmul N>=256 (fp32r rate)
                qsq = slice(qc * 512 + cq, (qc + 1) * 512)
                pss = psum_s.tile([128, 2, 512], F32, tag="s",
                                  name=f"pss{p}_{qc}_{ki}")
                nc.tensor.matmul(pss[:, 0, cq:], KT[0:64, ksl],
                                 QT[0:64, qsq], tile_position=(0, 0))
                nc.tensor.matmul(pss[:, 1, cq:], KT[64:128, ksl],
                                 QT[64:128, qsq], tile_position=(64, 0))
                se = s_p.tile([128, 2, 512], F32R, tag="se",
                              name=f"se{p}_{qc}_{ki}")
                nc.scalar.activation(se[:, :, ci:], pss[:, :, ci:],
                                     AF.Exp, scale=0.125)
                if ki >= 4 * qc:
                    tm = trimask[:].unsqueeze(1)
                    nc.vector.tensor_tensor(
                        out=se[:, :, ci:ci + 128], in0=se[:, :, ci:ci + 128],
                        in1=tm.broadcast_to([128, 2, 128]), op=OP.mult)
                nc.tensor.matmul(psA[:, ci:], V[:, ki, 0:DK + RO],
                                 se[:, 0, ci:],
                                 start=(ki == 0), stop=(ki == nki - 1))
                nc.tensor.matmul(psB[:, ci:], V[:, ki, DK + RO:VW],
                                 se[:, 1, ci:],
                                 start=(ki == 0), stop=(ki == nki - 1))
            # evacuate accumulators promptly, normalize from SBUF
            ct = bc_p.tile([DK + 1, 2, 512], F32, tag="ct",
                           name=f"ct{p}_{qc}")
            nc.vector.tensor_copy(ct[:, 0, :], psA[0:DK + 1, :])
            nc.vector.tensor_copy(ct[:, 1, :], psB[0:DK + 1, :])
            rec = rec_p.tile([1, 2, 512], F32, tag="rec", name=f"rec{p}_{qc}")
            nc.vector.reciprocal(rec[:], ct[64:65, :, :])
            bcr = bc_p.tile([64, 2, 512], F32, tag="bc", name=f"bc{p}_{qc}")
            nc.gpsimd.partition_broadcast(bcr[:], rec[:])
            nc.vector.tensor_tensor(out=ctx_sb[p][0:64, qsl],
                                    in0=ct[0:64, 0, :], in1=bcr[:, 0, :],
                                    op=OP.mult)
            nc.vector.tensor_tensor(out=ctx_sb[p][64:128, qsl],
                                    in0=ct[0:64, 1, :], in1=bcr[:, 1, :],
                                    op=OP.mult)

        def emit_wo_dma():
            for p in range(NPAIR):
                wo = wt_p.tile([128, D], F32R, tag="w", name=f"wo{p}")
                nc.sync.dma_start(
                    out=wo[:],
                    in_=wot_d[p * 128:(p + 1) * 128, :].bitcast(F32R))
                WO[p] = wo

        class Filler:
            def __init__(self, units):
                self._units = list(units)
                self._q = list(units)
            def pop(self):
                if self._q:
                    self._q.pop(0)()
            def flush(self):
                while self._q:
                    self._q.pop(0)()

        def outproj_units(qc):
            units = []
            for tt in range(4 * qc, 4 * (qc + 1)):
                for ec in range(2):
                    units.append(lambda tt=tt, ec=ec: _outproj_unit(tt, ec))
            return units

        def _outproj_unit(tt, ec):
            tsl = slice(tt * 128, (tt + 1) * 128)
            esl = slice(ec * 512, (ec + 1) * 512)
            pso = psum_mm.tile([128, 512], F32, tag="mm",
                               name=f"pso{tt}_{ec}")
            for p in range(NPAIR):
                nc.tensor.matmul(pso[:], ctx_sb[p][:, tsl],
                                 WO[p][:, esl],
                                 start=(p == 0), stop=(p == NPAIR - 1))
            st = st_p.tile([128, 512], F32, tag="st",
                           name=f"st{tt}_{ec}")
            nc.vector.tensor_copy(st[:], pso[:])
            nc.sync.dma_start(out=out_d[tsl, esl], in_=st[:])

        def emit_outproj(qc):
            """out tiles for the t-range of this q-chunk (needs all ctx)."""
            for tt in range(4 * qc, 4 * (qc + 1)):
                tsl = slice(tt * 128, (tt + 1) * 128)
                for ec in range(2):
                    esl = slice(ec * 512, (ec + 1) * 512)
                    pso = psum_mm.tile([128, 512], F32, tag="mm",
                                       name=f"pso{tt}_{ec}")
                    for p in range(NPAIR):
                        nc.tensor.matmul(pso[:], ctx_sb[p][:, tsl],
                                         WO[p][:, esl],
                                         start=(p == 0), stop=(p == NPAIR - 1))
                    st = st_p.tile([128, 512], F32, tag="st",
                                   name=f"st{tt}_{ec}")
                    nc.vector.tensor_copy(st[:], pso[:])
                    nc.sync.dma_start(out=out_d[tsl, esl], in_=st[:])

        # ---------- emission schedule ----------
        alloc_qkv(0)
        emit_a0_group(0)
        emit_w_dma(0)
        emit_qkv_piece(0, 0)
        emit_qkv_piece(0, 1)
        emit_a0_group(1)
        emit_qkv_piece(0, 2)
        emit_qkv_piece(0, 3)
        for p in range(NPAIR):
            if p < NPAIR - 1:
                emit_w_dma(p + 1)
                alloc_qkv(p + 1)
                for qc in range(4):
                    emit_attention(p, qc)
                    emit_qkv_piece(p + 1, qc)
            else:
                emit_wo_dma()
                for qc in range(4):
                    emit_attention(p, qc)
                    emit_outproj(qc)

    nc.compile()
    _nc_cache = nc
    return nc


def kernel(x, Wq, Wk, Wv, Wo):
    from concourse.bass_utils import run_bass_kernel_spmd

    nc = _build()
    x = np.asarray(x, dtype=np.float32)
    in_maps = []
    for c in range(NCORES):
        b, hg = c // 2, c % 2
        sl = slice(hg * ESH, (hg + 1) * ESH)
        in_maps.append({
            "x": np.ascontiguousarray(x[b]),
            "wqt": np.ascontiguousarray(np.asarray(Wq)[sl, :].T),
            "wkt": np.ascontiguousarray(np.asarray(Wk)[sl, :].T),
            "wvt": np.ascontiguousarray(np.asarray(Wv)[sl, :].T),
            "wot": np.ascontiguousarray(np.asarray(Wo)[:, sl].T),
        })
    res = run_bass_kernel_spmd(nc, in_maps, list(range(NCORES)))
    outs = [res.results[c]["out"] for c in range(NCORES)]
    return np.stack([outs[2 * b] + outs[2 * b + 1] for b in range(B)])


# revision 24
# speedup vs baseline: 1.0515x; 1.0515x over previous
"""Self-contained MHA kernel for Trainium2, 8 NeuronCores.

Problem: B=4, T=2048, D=1024, H=16 causal MHA, fp32, no bias.
Sharding: core c handles batch b=c//2 and head-group hg=c%2 (8 heads each),
Megatron-style: Wq/Wk/Wv column-sharded, Wo row-sharded; host sums the two
partial outputs per batch.

Per-core pipeline (emission interleaved so ACT-bound attention overlaps
PE-bound projections of the next head-pair):
 - PE-transpose x -> xT [d, t] (f32r), split in two t-group tiles
 - weight-stationary projections: QT/KT [2 heads on partitions, t],
   V via V^T + PE transpose, stored as [t, A|ones|B|ones] (double-buffered)
 - attention per head-pair, transposed scores S^T[k, q] = K Q^T
   (row-packed 2 heads per PE pass), exp on ScalarE with scale=1/8,
   no max subtraction (scores ~ N(0,1)); fully-masked columns skipped,
   triangular mask multiply only on the diagonal 128-col block
 - ctx^T[dv,q] + softmax denominator in one matmul: lhsT=[V|ones]
 - prompt PSUM evacuation, then reciprocal + partition_broadcast + multiply
 - out projection: lhsT=ctx tiles, rhs=WoT -> out[t, e] partial, interleaved
   with the last pair's attention
"""

import numpy as np

B, T, D, H = 4, 2048, 1024, 16
DK = 64
NCORES = 8
NPAIR = 4        # head-pairs per core
ESH = 512        # output-feature shard per core (8 heads * 64)
RO = 4           # ones columns appended per head in the V stationary
VW = 2 * (DK + RO)   # 136 cols per t-tile in V

_nc_cache = None


def _build():
    global _nc_cache
    if _nc_cache is not None:
        return _nc_cache

    from contextlib import ExitStack

    import concourse.bacc as bacc
    import concourse.mybir as mybir
    import concourse.tile as tile
    from concourse.masks import make_identity, make_upper_triangular

    F32 = mybir.dt.float32
    F32R = mybir.dt.float32r
    AF = mybir.ActivationFunctionType
    OP = mybir.AluOpType

    nc = bacc.Bacc("TRN2", target_bir_lowering=False, debug=False,
                   num_devices=NCORES)
    x_d = nc.declare_dram_parameter("xt", [D, T], F32, isOutput=False)
    wqt_d = nc.declare_dram_parameter("wqt", [D, ESH], F32, isOutput=False)
    wkt_d = nc.declare_dram_parameter("wkt", [D, ESH], F32, isOutput=False)
    wvt_d = nc.declare_dram_parameter("wvt", [D, ESH], F32, isOutput=False)
    wot_d = nc.declare_dram_parameter("wot", [ESH, D], F32, isOutput=False)
    out_d = nc.declare_dram_parameter("out", [T, D], F32, isOutput=True)

    with tile.TileContext(nc) as tc, ExitStack() as ctx:
        const_p = ctx.enter_context(tc.tile_pool(name="const", bufs=1))
        xt_p = ctx.enter_context(tc.tile_pool(name="xt", bufs=1))
        wt_p = ctx.enter_context(tc.tile_pool(name="wt", bufs=4))
        qt_p = ctx.enter_context(tc.tile_pool(name="qt", bufs=2))
        kt_p = ctx.enter_context(tc.tile_pool(name="kt", bufs=2))
        v_pool = ctx.enter_context(tc.tile_pool(name="vp", bufs=2))
        vt_p = ctx.enter_context(tc.tile_pool(name="vtmp", bufs=1))
        ctx_p = ctx.enter_context(tc.tile_pool(name="ctxp", bufs=1))
        s_p = ctx.enter_context(tc.tile_pool(name="sp", bufs=4))
        st_p = ctx.enter_context(tc.tile_pool(name="stage", bufs=2))
        bc_p = ctx.enter_context(tc.tile_pool(name="bc", bufs=1))
        rec_p = ctx.enter_context(tc.tile_pool(name="rec", bufs=2))
        psum_s = ctx.enter_context(tc.tile_pool(name="psum_s", bufs=2,
                                                space="PSUM"))
        psum_ctx = ctx.enter_context(tc.tile_pool(name="psum_ctx", bufs=2,
                                                  space="PSUM"))
        psum_mm = ctx.enter_context(tc.tile_pool(name="psum_mm", bufs=1,
                                                 space="PSUM"))

        # constants
        ident = const_p.tile([128, 128], F32)
        make_identity(nc, ident[:])
        trimask = const_p.tile([128, 128], F32)
        make_upper_triangular(nc, trimask[:], val=1.0, diag=True)
        ones32 = const_p.tile([128, RO], F32)
        nc.gpsimd.memset(ones32[:], 1.0)

        # xT split per t-group for finer dependencies
        xT = [xt_p.tile([128, 8, 1024], F32R, tag=f"xt{g}", name=f"xT{g}")
              for g in range(2)]
        ctx_sb = [ctx_p.tile([128, T], F32R, tag=f"ctx{p}", name=f"ctx_sb{p}")
                  for p in range(NPAIR)]

        W_TILES = {}   # p -> (wq, wk, wv)
        QKV = {}       # p -> (QT, KT, V)
        WO = {}
        VT_TMP = {}

        def emit_a0_group(tg):
            """DMA pre-transposed x into xT for t-group tg."""
            tsl = slice(tg * 1024, (tg + 1) * 1024)
            for dt_ in range(8):
                nc.sync.dma_start(
                    out=xT[tg][:, dt_, :],
                    in_=x_d[dt_ * 128:(dt_ + 1) * 128, tsl].bitcast(F32R))

        def emit_w_dma(p):
            wq = wt_p.tile([128, 8, 128], F32R, tag="w", name=f"wq{p}")
            wk = wt_p.tile([128, 8, 128], F32R, tag="w", name=f"wk{p}")
            wv = wt_p.tile([128, 8, 128], F32R, tag="w", name=f"wv{p}")
            esl = slice(p * 128, (p + 1) * 128)
            for dt_ in range(8):
                dsl = slice(dt_ * 128, (dt_ + 1) * 128)
                nc.sync.dma_start(out=wq[:, dt_, :],
                                  in_=wqt_d[dsl, esl].bitcast(F32R))
                nc.sync.dma_start(out=wk[:, dt_, :],
                                  in_=wkt_d[dsl, esl].bitcast(F32R))
                nc.sync.dma_start(out=wv[:, dt_, :],
                                  in_=wvt_d[dsl, esl].bitcast(F32R))
            W_TILES[p] = (wq, wk, wv)

        def alloc_qkv(p):
            QT = qt_p.tile([128, T], F32R, tag="q", name=f"QT{p}")
            KT = kt_p.tile([128, T], F32R, tag="k", name=f"KT{p}")
            V = v_pool.tile([128, 16, VW], F32R, tag="v", name=f"V{p}")
            QKV[p] = (QT, KT, V)

        def _qkv_mm_unit(p, tg, which):
            """one 16-MM projection group + evac (which in 'q','k','v')."""
            wq, wk, wv = W_TILES[p]
            QT, KT, V = QKV[p]
            w_sb, dst = {"q": (wq, QT), "k": (wk, KT), "v": (wv, None)}[which]
            ps = psum_mm.tile([128, 2, 512], F32, tag="mm",
                              name=f"ps{p}_{tg}_{which}")
            for half in range(2):
                csl = slice(half * 512, (half + 1) * 512)
                for dt_ in range(8):
                    nc.tensor.matmul(ps[:, half, :], w_sb[:, dt_, :],
                                     xT[tg][:, dt_, csl],
                                     start=(dt_ == 0), stop=(dt_ == 7))
            if which == "v":
                vtmp = vt_p.tile([128, 1024], F32, tag="vt",
                                 name=f"vtmp{p}_{tg}")
                nc.vector.tensor_copy(vtmp[:],
                                      ps[:].rearrange("p a b -> p (a b)"))
                VT_TMP[(p, tg)] = vtmp
            else:
                nc.vector.tensor_copy(
                    dst[:, tg * 1024:(tg + 1) * 1024],
                    ps[:].rearrange("p a b -> p (a b)"))

        def _v_tr_unit(p, tg, vg):
            """4 V transposes + layout copies."""
            V = QKV[p][2]
            vtmp = VT_TMP[(p, tg)]
            pst = psum_s.tile([128, 4, 128], F32, tag="s",
                              name=f"pstv{p}_{tg}_{vg}")
            for j in range(4):
                sub = vg * 4 + j
                nc.tensor.matmul(
                    pst[:, j, :],
                    vtmp[:, sub * 128:(sub + 1) * 128],
                    ident[:], is_transpose=True,
                    start=(j == 0), stop=(j == 3))
            gts = slice(tg * 8 + vg * 4, tg * 8 + (vg + 1) * 4)
            nc.vector.tensor_copy(V[:, gts, 0:DK], pst[:, :, 0:DK])
            nc.vector.tensor_copy(V[:, gts, DK + RO:DK + RO + DK],
                                  pst[:, :, DK:128])
            nc.vector.tensor_copy(
                V[:, gts, DK:DK + RO],
                ones32[:].unsqueeze(1).broadcast_to([128, 4, RO]))
            nc.vector.tensor_copy(
                V[:, gts, DK + RO + DK:VW],
                ones32[:].unsqueeze(1).broadcast_to([128, 4, RO]))

        def qkv_units(p):
            units = []
            for tg in range(2):
                for which in ("q", "k", "v"):
                    units.append(lambda p=p, tg=tg, w=which: _qkv_mm_unit(p, tg, w))
                for vg in range(2):
                    units.append(lambda p=p, tg=tg, vg=vg: _v_tr_unit(p, tg, vg))
            return units

        def emit_qkv_piece(p, piece):
            """piece 0..3: (tg0:QK, tg0:V, tg1:QK, tg1:V)."""
            tg, kind = divmod(piece, 2)
            if kind == 0:
                _qkv_mm_unit(p, tg, "q")
                _qkv_mm_unit(p, tg, "k")
            else:
                _qkv_mm_unit(p, tg, "v")
                _v_tr_unit(p, tg, 0)
                _v_tr_unit(p, tg, 1)

        def emit_qkv_chunk(p, ch):
            """512-col chunk granularity (used for pair 0 startup)."""
            wq, wk, wv = W_TILES[p]
            QT, KT, V = QKV[p]
            tg, half = divmod(ch, 2)
            csl = slice(half * 512, (half + 1) * 512)
            osl = slice(ch * 512, (ch + 1) * 512)
            for w_sb, dst in ((wq, QT), (wk, KT)):
                ps = psum_mm.tile([128, 512], F32, tag="mm",
                                  name=f"psc{p}_{ch}_{dst.name}")
                for dt_ in range(8):
                    nc.tensor.matmul(ps[:], w_sb[:, dt_, :],
                                     xT[tg][:, dt_, csl],
                                     start=(dt_ == 0), stop=(dt_ == 7))
                nc.vector.tensor_copy(dst[:, osl], ps[:])
            psv = psum_mm.tile([128, 512], F32, tag="mm", name=f"psvc{p}_{ch}")
            for dt_ in range(8):
                nc.tensor.matmul(psv[:], wv[:, dt_, :], xT[tg][:, dt_, csl],
                                 start=(dt_ == 0), stop=(dt_ == 7))
            vtmp = vt_p.tile([128, 512], F32, tag="vt", name=f"vtc{p}_{ch}")
            nc.vector.tensor_copy(vtmp[:], psv[:])
            pst = psum_s.tile([128, 4, 128], F32, tag="s", name=f"pstc{p}_{ch}")
            for j in range(4):
                nc.tensor.matmul(pst[:, j, :],
                                 vtmp[:, j * 128:(j + 1) * 128],
                                 ident[:], is_transpose=True,
                                 start=(j == 0), stop=(j == 3))
            gts = slice(ch * 4, (ch + 1) * 4)
            nc.vector.tensor_copy(V[:, gts, 0:DK], pst[:, :, 0:DK])
            nc.vector.tensor_copy(V[:, gts, DK + RO:DK + RO + DK],
                                  pst[:, :, DK:128])
            nc.vector.tensor_copy(
                V[:, gts, DK:DK + RO],
                ones32[:].unsqueeze(1).broadcast_to([128, 4, RO]))
            nc.vector.tensor_copy(
                V[:, gts, DK + RO + DK:VW],
                ones32[:].unsqueeze(1).broadcast_to([128, 4, RO]))

        def emit_attention(p, qc, filler=None):
            QT, KT, V = QKV[p]
            qsl = slice(qc * 512, (qc + 1) * 512)
            nki = 4 * (qc + 1)
            psA = psum_ctx.tile([DK + RO, 512], F32, tag="ctx",
                                name=f"psA{p}_{qc}")
            psB = psum_ctx.tile([DK + RO, 512], F32, tag="ctx",
                                name=f"psB{p}_{qc}")
            for ki in range(nki):
                if filler and ki > 0 and (ki * len(filler._units)) % nki == 0:
                    filler.pop()
                ksl = slice(ki * 128, (ki + 1) * 128)
                ci = max(0, (ki - 4 * qc) * 128)
                cq = min(ci, 256)     # keep QK matmul N>=256 (fp32r rate)
                qsq = slice(qc * 512 + cq, (qc + 1) * 512)
                pss = psum_s.tile([128, 2, 512], F32, tag="s",
                                  name=f"pss{p}_{qc}_{ki}")
                nc.tensor.matmul(pss[:, 0, cq:], KT[0:64, ksl],
                                 QT[0:64, qsq], tile_position=(0, 0))
                nc.tensor.matmul(pss[:, 1, cq:], KT[64:128, ksl],
                                 QT[64:128, qsq], tile_position=(64, 0))
                se = s_p.tile([128, 2, 512], F32R, tag="se",
                              name=f"se{p}_{qc}_{ki}")
                nc.scalar.activation(se[:, :, ci:], pss[:, :, ci:],
                                     AF.Exp, scale=0.125)
                if ki >= 4 * qc:
                    tm = trimask[:].unsqueeze(1)
                    nc.vector.tensor_tensor(
                        out=se[:, :, ci:ci + 128], in0=se[:, :, ci:ci + 128],
                        in1=tm.broadcast_to([128, 2, 128]), op=OP.mult)
                nc.tensor.matmul(psA[:, ci:], V[:, ki, 0:DK + RO],
                                 se[:, 0, ci:],
                                 start=(ki == 0), stop=(ki == nki - 1))
                nc.tensor.matmul(psB[:, ci:], V[:, ki, DK + RO:VW],
                                 se[:, 1, ci:],
                                 start=(ki == 0), stop=(ki == nki - 1))
            # evacuate accumulators promptly, normalize from SBUF
            ct = bc_p.tile([DK + 1, 2, 512], F32, tag="ct",
                           name=f"ct{p}_{qc}")
            nc.vector.tensor_copy(ct[:, 0, :], psA[0:DK + 1, :])
            nc.vector.tensor_copy(ct[:, 1, :], psB[0:DK + 1, :])
            rec = rec_p.tile([1, 2, 512], F32, tag="rec", name=f"rec{p}_{qc}")
            nc.vector.reciprocal(rec[:], ct[64:65, :, :])
            bcr = bc_p.tile([64, 2, 512], F32, tag="bc", name=f"bc{p}_{qc}")
            nc.gpsimd.partition_broadcast(bcr[:], rec[:])
            nc.vector.tensor_tensor(out=ctx_sb[p][0:64, qsl],
                                    in0=ct[0:64, 0, :], in1=bcr[:, 0, :],
                                    op=OP.mult)
            nc.vector.tensor_tensor(out=ctx_sb[p][64:128, qsl],
                                    in0=ct[0:64, 1, :], in1=bcr[:, 1, :],
                                    op=OP.mult)

        def emit_wo_dma():
            for p in range(NPAIR):
                wo = wt_p.tile([128, D], F32R, tag="w", name=f"wo{p}")
                nc.sync.dma_start(
                    out=wo[:],
                    in_=wot_d[p * 128:(p + 1) * 128, :].bitcast(F32R))
                WO[p] = wo

        class Filler:
            def __init__(self, units):
                self._units = list(units)
                self._q = list(units)
            def pop(self):
                if self._q:
                    self._q.pop(0)()
            def flush(self):
                while self._q:
                    self._q.pop(0)()

        def outproj_units(qc):
            units = []
            for tt in range(4 * qc, 4 * (qc + 1)):
                for ec in range(2):
                    units.append(lambda tt=tt, ec=ec: _outproj_unit(tt, ec))
            return units

        def _outproj_unit(tt, ec):
            tsl = slice(tt * 128, (tt + 1) * 128)
            esl = slice(ec * 512, (ec + 1) * 512)
            pso = psum_mm.tile([128, 512], F32, tag="mm",
                               name=f"pso{tt}_{ec}")
            for p in range(NPAIR):
                nc.tensor.matmul(pso[:], ctx_sb[p][:, tsl],
                                 WO[p][:, esl],
                                 start=(p == 0), stop=(p == NPAIR - 1))
            st = st_p.tile([128, 512], F32, tag="st",
                           name=f"st{tt}_{ec}")
            nc.vector.tensor_copy(st[:], pso[:])
            nc.sync.dma_start(out=out_d[tsl, esl], in_=st[:])

        def emit_outproj(qc):
            """out tiles for the t-range of this q-chunk (needs all ctx)."""
            for tt in range(4 * qc, 4 * (qc + 1)):
                tsl = slice(tt * 128, (tt + 1) * 128)
                for ec in range(2):
                    esl = slice(ec * 512, (ec + 1) * 512)
                    pso = psum_mm.tile([128, 512], F32, tag="mm",
                                       name=f"pso{tt}_{ec}")
                    for p in range(NPAIR):
                        nc.tensor.matmul(pso[:], ctx_sb[p][:, tsl],
                                         WO[p][:, esl],
                                         start=(p == 0), stop=(p == NPAIR - 1))
                    st = st_p.tile([128, 512], F32, tag="st",
                                   name=f"st{tt}_{ec}")
                    nc.vector.tensor_copy(st[:], pso[:])
                    nc.sync.dma_start(out=out_d[tsl, esl], in_=st[:])

        # ---------- emission schedule ----------
        alloc_qkv(0)
        emit_a0_group(0)
        emit_w_dma(0)
        emit_qkv_piece(0, 0)
        emit_qkv_piece(0, 1)
        emit_a0_group(1)
        emit_qkv_piece(0, 2)
        emit_qkv_piece(0, 3)
        for p in range(NPAIR):
            if p < NPAIR - 1:
                emit_w_dma(p + 1)
                alloc_qkv(p + 1)
                for qc in range(4):
                    emit_attention(p, qc)
                    emit_qkv_piece(p + 1, qc)
            else:
                emit_wo_dma()
                for qc in range(4):
                    emit_attention(p, qc)
                    emit_outproj(qc)

    nc.compile()
    _nc_cache = nc
    return nc


def kernel(x, Wq, Wk, Wv, Wo):
    from concourse.bass_utils import run_bass_kernel_spmd

    nc = _build()
    x = np.asarray(x, dtype=np.float32)
    in_maps = []
    for c in range(NCORES):
        b, hg = c // 2, c % 2
        sl = slice(hg * ESH, (hg + 1) * ESH)
        in_maps.append({
            "xt": np.ascontiguousarray(x[b].T),
            "wqt": np.ascontiguousarray(np.asarray(Wq)[sl, :].T),
            "wkt": np.ascontiguousarray(np.asarray(Wk)[sl, :].T),
            "wvt": np.ascontiguousarray(np.asarray(Wv)[sl, :].T),
            "wot": np.ascontiguousarray(np.asarray(Wo)[:, sl].T),
        })
    res = run_bass_kernel_spmd(nc, in_maps, list(range(NCORES)))
    outs = [res.results[c]["out"] for c in range(NCORES)]
    return np.stack([outs[2 * b] + outs[2 * b + 1] for b in range(B)])
